# revision 6
# baseline (speedup 1.0000x reference)
"""2-layer GraphSAGE (mean aggregation) on 8 Trainium2 NeuronCores — v2.

Key differences vs v1 (see git/_transcript history):
- fp16 data path everywhere (tables, staging, indicator, weights): halves
  gather bytes, 4x faster PE matmuls vs fp32, 2x DVE mode.
- Flipped aggregation matmul: accT[f, dst] += stage[pos, f]^T @ ind[pos, dst]
  with 1/cnt folded into the indicator ((iota==seg)*rscale on DVE, one op).
  Eliminates the mean-scale copy and both PE transposes of v1.
- Self terms via SBUF-resident transposed tables (x_shardT shipped from host,
  h1T written during layer 1) — no per-block DMA, no transposes.
- Gather calls span a whole (superblock=8 blocks, chunk) run (~4K indices)
  instead of <=512: amortizes the ~1us SWDGE fixed cost per call.
- h1 exchange: fp16 AllGather in AGP contiguous piece-major slabs that
  overlap under the layer-1 tail. h1 rows are stored 128-wide fp16 (256B,
  the gather stride granule; upper half garbage, never consumed) so layer-2
  gathers run directly on the exchanged table with no expand pass.
"""
import sys
sys.path.insert(0, "/opt/trn_rl_repo")
import numpy as np

import concourse.bass as bass
import concourse.bacc as bacc
import concourse.mybir as mybir
import concourse.tile as tile
from concourse.ap import AP
from concourse.bass_utils import run_bass_kernel_spmd
from concourse.masks import make_identity

N_NODES = 100000
N_EDGES = 1600000
F_IN = 128
F_OUT = 64
P = 8
NREAL = 12500
NL = 12544            # padded dsts per core (112*112)
BLK = 112             # dsts per block
NB = NL // BLK        # 112 blocks
SBB = 8               # blocks per superblock
NSB = NB // SBB       # 14 superblocks
NG = P * NL           # 100352 padded global rows
CHUNK = 32768
NCHUNK = (NG + CHUNK - 1) // CHUNK   # 4
MAXI = 1024           # max idxs per dma_gather call (single_packet sweet spot)
AGP = 4               # AllGather pieces (piece-major h1 table layout)
PBLK = NB // AGP      # blocks per piece
PROWS = PBLK * BLK    # rows per piece per core


def _wrap16(flat_idx):
    w = flat_idx.reshape(-1, 16).T.copy()
    return np.tile(w, (8, 1))


def _pm_row(slot, core):
    """piece-major row id of node (core, slot) in the h1 table."""
    g = slot // PROWS
    return g * (P * PROWS) + core * PROWS + (slot - g * PROWS)


def _layer_structure(rowid, dcore, dslot, cnt_all):
    """Host-side layout for one layer's gathers.

    rowid: per-edge row index into this layer's table (global, < NG).
    Returns dict with call plan + per-core idx/seg/rsc streams + tile maps.
    """
    blk = dslot // BLK
    m = dslot % BLK
    sb = blk // SBB
    ch = rowid // CHUNK

    order = np.lexsort((rowid, blk, ch, sb, dcore))
    dc_o = dcore[order]
    sb_o = sb[order]
    ch_o = ch[order]
    bl_o = blk[order]
    m_o = m[order]
    rid_o = rowid[order]
    dslot_o = dslot[order]

    # counts per (core, block, chunk)
    key = (dc_o * NB + bl_o) * NCHUNK + ch_o
    counts = np.bincount(key, minlength=P * NB * NCHUNK).reshape(P, NB, NCHUNK)
    ncols_u = np.maximum((counts + 127) // 128, 0).max(axis=0)  # [NB, NCHUNK]
    # ensure at least the columns exist when any core has edges; zero-col runs ok

    # column layout: sb -> chunk -> block
    col_of_bc = np.zeros((NB, NCHUNK), dtype=np.int64)
    runs = {}           # (sb, c) -> (start_col, ncols)
    col = 0
    for s in range(NSB):
        for c in range(NCHUNK):
            start = col
            for b in range(s * SBB, (s + 1) * SBB):
                col_of_bc[b, c] = col
                col += int(ncols_u[b, c])
            runs[(s, c)] = (start, col - start)
    TC = col

    # call plan: per (sb, c) run split into pieces <= MAXI//128 cols
    calls = []   # (sb, c, start_col, piece_cols)
    for s in range(NSB):
        for c in range(NCHUNK):
            start, n = runs[(s, c)]
            done = 0
            while done < n:
                piece = min(n - done, MAXI // 128)
                calls.append((s, c, start + done, piece))
                done += piece

    # per-core streams
    idx_cores, seg_cores, rsc_cores = [], [], []
    # group start/end in sorted stream per (core,b,c)
    diff = np.flatnonzero(np.diff(key)) + 1
    starts = np.concatenate([[0], diff])
    ends = np.concatenate([diff, [len(key)]])
    for k in range(P):
        idx_flat = np.zeros(TC * 128, dtype=np.int16)
        seg_flat = np.zeros(TC * 128, dtype=np.float32)
        rsc_flat = np.zeros(TC * 128, dtype=np.float32)
        rcnt = 1.0 / np.maximum(cnt_all[k], 1.0)   # [NL]
        for s_i, e_i in zip(starts, ends):
            if dc_o[s_i] != k:
                continue
            b, c = int(bl_o[s_i]), int(ch_o[s_i])
            n = e_i - s_i
            pos0 = int(col_of_bc[b, c]) * 128
            idx_flat[pos0:pos0 + n] = (rid_o[s_i:e_i] - c * CHUNK).astype(np.int16)
            seg_flat[pos0:pos0 + n] = m_o[s_i:e_i]
            rsc_flat[pos0:pos0 + n] = rcnt[dslot_o[s_i:e_i]]
        idx_cores.append(_wrap16(idx_flat))
        seg_cores.append(seg_flat.reshape(TC, 128).T.copy())
        rsc_cores.append(rsc_flat.reshape(TC, 128).T.copy())

    return dict(ncols_u=ncols_u, col_of_bc=col_of_bc, runs=runs, calls=calls,
                TC=TC, idx=idx_cores, seg=seg_cores, rsc=rsc_cores)


def _preprocess(edge_index):
    src = np.asarray(edge_index[0], dtype=np.int64)
    dst = np.asarray(edge_index[1], dtype=np.int64)
    dcore = dst // NREAL
    dslot = dst - dcore * NREAL
    score = src // NREAL
    sslot = src - score * NREAL

    # per-core dst degree counts (for mean)
    cnt_all = []
    for k in range(P):
        sel = dcore == k
        cnt_all.append(np.bincount(dslot[sel], minlength=NL).astype(np.float32))

    rowid1 = score * NL + sslot                       # slot-major x table
    g = sslot // PROWS                                # piece-major h1 table
    rowid2 = g * (P * PROWS) + score * PROWS + (sslot - g * PROWS)

    L1 = _layer_structure(rowid1, dcore, dslot, cnt_all)
    L2 = _layer_structure(rowid2, dcore, dslot, cnt_all)
    return dict(L1=L1, L2=L2)


def _build(meta, mode="full", rep=1):
    L = {1: meta["L1"], 2: meta["L2"]}
    TC1, TC2 = L[1]["TC"], L[2]["TC"]

    nc = bacc.Bacc("TRN2", target_bir_lowering=False, debug=False,
                   num_devices=P, num_swdge_queues=4)
    dt = mybir.dt
    f16, f32 = dt.float16, dt.float32

    x_fullh = nc.dram_tensor("x_fullh", [NG, F_IN], f16, kind="ExternalInput")
    xT_d = nc.dram_tensor("xT", [F_IN, NL], f16, kind="ExternalInput")
    idx1_d = nc.dram_tensor("idx1", [128, TC1 * 8], dt.int16, kind="ExternalInput")
    seg1_d = nc.dram_tensor("seg1", [128, TC1], f32, kind="ExternalInput")
    rsc1_d = nc.dram_tensor("rsc1", [128, TC1], f32, kind="ExternalInput")
    idx2_d = nc.dram_tensor("idx2", [128, TC2 * 8], dt.int16, kind="ExternalInput")
    seg2_d = nc.dram_tensor("seg2", [128, TC2], f32, kind="ExternalInput")
    rsc2_d = nc.dram_tensor("rsc2", [128, TC2], f32, kind="ExternalInput")
    iota_d = nc.dram_tensor("iota", [128, BLK], f16, kind="ExternalInput")
    wl1_d = nc.dram_tensor("W_l1", [F_IN, F_OUT], f16, kind="ExternalInput")
    wr1_d = nc.dram_tensor("W_r1", [F_IN, F_OUT], f16, kind="ExternalInput")
    b1_d = nc.dram_tensor("b1", [1, F_OUT], f16, kind="ExternalInput")
    wl2_d = nc.dram_tensor("W_l2", [F_OUT, F_OUT], f16, kind="ExternalInput")
    wr2_d = nc.dram_tensor("W_r2", [F_OUT, F_OUT], f16, kind="ExternalInput")
    b2_d = nc.dram_tensor("b2", [1, F_OUT], f16, kind="ExternalInput")
    out_d = nc.dram_tensor("out", [NL, F_OUT], f32, kind="ExternalOutput")

    # internal DRAM
    h1_piece = [nc.dram_tensor(f"h1p{g}", [PROWS, F_IN], f16) for g in range(AGP)]
    h1_full = nc.dram_tensor("h1_full", [NG, F_IN], f16, addr_space="Shared")

    ident = mybir.ActivationFunctionType
    AOT = mybir.AluOpType

    # max stage cols over (sb, c) runs, both layers
    max_run = max(
        max(n for (_, n) in L[1]["runs"].values()),
        max(n for (_, n) in L[2]["runs"].values()),
    )

    with tile.TileContext(nc) as tc:
        with (
            tc.tile_pool(name="const", bufs=1) as constp,
            tc.tile_pool(name="tabs", bufs=1) as tabp,
            tc.tile_pool(name="stagep", bufs=8) as stagep,
            tc.tile_pool(name="indp", bufs=16) as indp,
            tc.tile_pool(name="op", bufs=8) as op,
            tc.tile_pool(name="ps_acc", bufs=4, space="PSUM") as ps_acc,
            tc.tile_pool(name="ps_o", bufs=2, space="PSUM") as ps_o,
            tc.tile_pool(name="ps_t", bufs=2, space="PSUM") as ps_t,
        ):
            iota_t = constp.tile([128, BLK], f16)
            nc.sync.dma_start(iota_t[:], iota_d[:])
            xT_t = constp.tile([F_IN, NL], f16)
            nc.sync.dma_start(xT_t[:], xT_d[:])
            h1T_t = constp.tile([F_OUT, NL], f16)
            wl1_t = constp.tile([F_IN, F_OUT], f16)
            nc.sync.dma_start(wl1_t[:], wl1_d[:])
            wr1_t = constp.tile([F_IN, F_OUT], f16)
            nc.sync.dma_start(wr1_t[:], wr1_d[:])
            wl2_t = constp.tile([F_OUT, F_OUT], f16)
            nc.sync.dma_start(wl2_t[:], wl2_d[:])
            wr2_t = constp.tile([F_OUT, F_OUT], f16)
            nc.sync.dma_start(wr2_t[:], wr2_d[:])
            b1_t = constp.tile([1, F_OUT], f16)
            nc.sync.dma_start(b1_t[:], b1_d[:])
            b2_t = constp.tile([1, F_OUT], f16)
            nc.sync.dma_start(b2_t[:], b2_d[:])
            ones_t = constp.tile([1, BLK], f16)
            nc.vector.memset(ones_t[:], 1.0)
            id_t = constp.tile([BLK, BLK], f16)
            make_identity(nc, id_t[:])

            qn = [0]

            def load_tables(li):
                S = L[li]
                TC = S["TC"]
                idx_t = tabp.tile([128, max(TC1, TC2) * 8], dt.int16, tag="idx")
                seg_t = tabp.tile([128, max(TC1, TC2)], f32, tag="seg")
                rsc_t = tabp.tile([128, max(TC1, TC2)], f32, tag="rsc")
                idx_d = idx1_d if li == 1 else idx2_d
                seg_d = seg1_d if li == 1 else seg2_d
                rsc_d = rsc1_d if li == 1 else rsc2_d
                nc.sync.dma_start(idx_t[:, :TC * 8], idx_d[:])
                nc.sync.dma_start(seg_t[:, :TC], seg_d[:])
                nc.sync.dma_start(rsc_t[:, :TC], rsc_d[:])
                return idx_t, seg_t, rsc_t

            def gather_in_ap(li, c):
                lo = c * CHUNK
                hi = min((c + 1) * CHUNK, NG)
                tab = x_fullh if li == 1 else h1_full
                return tab[lo:hi, :], F_IN, None

            def emit_cc(g):
                nc.gpsimd.collective_compute(
                    "AllGather", AOT.bypass,
                    replica_groups=[list(range(P))],
                    ins=[h1_piece[g][:]],
                    outs=[h1_full[g * P * PROWS:(g + 1) * P * PROWS, :]])

            def layer(li, idx_t, seg_t, rsc_t, FW, wl_t, wr_t, bias_t, selfT,
                      gathers_only=False):
                S = L[li]
                ncols_u = S["ncols_u"]
                col_of_bc = S["col_of_bc"]
                runs = S["runs"]
                calls_by = {}
                for (s, c, start, piece) in S["calls"]:
                    calls_by.setdefault((s, c), []).append((start, piece))

                pending_cc = {}
                for s in range(NSB):
                    if li == 1:
                        for g in [g for g, due in list(pending_cc.items()) if due <= s]:
                            emit_cc(g)
                            del pending_cc[g]
                    stages = {}
                    for c in range(NCHUNK):
                        start, n = runs[(s, c)]
                        if n == 0:
                            continue
                        st = stagep.tile([128, max_run * F_IN], f16, tag="stage")
                        stages[c] = (st, start)
                        in_ap, esz, estep = gather_in_ap(li, c)
                        for (cst, piece) in calls_by[(s, c)]:
                            loc = cst - start
                            kw = dict(elem_size=esz, single_packet=(piece * 128 <= 1024),
                                      queue_num=qn[0] % 4)
                            if estep is not None:
                                kw["elem_step"] = estep
                            nc.gpsimd.dma_gather(
                                out_ap=st[:, loc * F_IN:(loc + piece) * F_IN]
                                    .rearrange("p (c f) -> p c f", f=F_IN),
                                in_ap=in_ap,
                                idxs_ap=idx_t[:, cst * 8:(cst + piece) * 8],
                                num_idxs=piece * 128, num_idxs_reg=piece * 128,
                                **kw)
                            qn[0] += 1
                    if gathers_only:
                        continue
                    for b in range(s * SBB, (s + 1) * SBB):
                        Tb = int(ncols_u[b, :].sum())
                        if Tb == 0:
                            continue
                        acc = ps_acc.tile([FW, BLK], f32, tag="acc")
                        t_i = 0
                        for c in range(NCHUNK):
                            ncc = int(ncols_u[b, c])
                            if ncc == 0:
                                continue
                            st, start = stages[c]
                            loc0 = int(col_of_bc[b, c]) - start
                            for j in range(ncc):
                                gcol = int(col_of_bc[b, c]) + j
                                ind = indp.tile([128, BLK], f16, tag="ind")
                                nc.vector.tensor_scalar(
                                    out=ind[:], in0=iota_t[:],
                                    scalar1=seg_t[:, gcol:gcol + 1],
                                    scalar2=rsc_t[:, gcol:gcol + 1],
                                    op0=AOT.is_equal, op1=AOT.mult)
                                nc.tensor.matmul(
                                    acc[:],
                                    lhsT=st[:, (loc0 + j) * F_IN:(loc0 + j) * F_IN + FW],
                                    rhs=ind[:],
                                    start=(t_i == 0), stop=(t_i == Tb - 1))
                                t_i += 1
                        aggT = op.tile([FW, BLK], f16, tag="aggT")
                        nc.scalar.activation(out=aggT[:], in_=acc[:], func=ident.Copy)
                        o_ps = ps_o.tile([BLK, F_OUT], f32, tag="ops")
                        nc.tensor.matmul(o_ps[:], lhsT=aggT[:], rhs=wl_t[:],
                                         start=True, stop=False)
                        nc.tensor.matmul(o_ps[:], lhsT=selfT[:, b * BLK:(b + 1) * BLK],
                                         rhs=wr_t[:], start=False, stop=False)
                        nc.tensor.matmul(o_ps[:], lhsT=ones_t[:1, :BLK], rhs=bias_t[:],
                                         start=False, stop=True)
                        if li == 1:
                            h1row = op.tile([BLK, F_OUT], f16, tag="h1row")
                            nc.scalar.activation(out=h1row[:], in_=o_ps[:],
                                                 func=ident.Relu)
                            g = b // PBLK
                            r0 = b * BLK - g * PROWS
                            nc.sync.dma_start(
                                h1_piece[g][r0:r0 + BLK, :F_OUT], h1row[:])
                            tps = ps_t.tile([F_OUT, BLK], f16, tag="tps")
                            nc.tensor.transpose(out=tps[:], in_=h1row[:],
                                                identity=id_t[:])
                            nc.scalar.activation(
                                out=h1T_t[:, b * BLK:(b + 1) * BLK], in_=tps[:],
                                func=ident.Copy)
                        else:
                            ob = op.tile([BLK, F_OUT], f32, tag="ob")
                            nc.scalar.activation(out=ob[:], in_=o_ps[:],
                                                 func=ident.Copy)
                            nc.sync.dma_start(out_d[b * BLK:(b + 1) * BLK, :], ob[:])
                        # queue AllGather piece; emitted 2 superblocks later so
                        # the in-order Pool sequencer never stalls on its deps
                        if li == 1 and mode != "l1" and (b + 1) % PBLK == 0:
                            g = b // PBLK
                            pending_cc[g] = (b + 1 + SBB - 1) // SBB + 2
                if li == 1 and mode != "l1":
                    for g in sorted(pending_cc):
                        emit_cc(g)
                    pending_cc.clear()

            for _r in range(rep):
                idx_t, seg_t, rsc_t = load_tables(1)
                layer(1, idx_t, seg_t, rsc_t, F_IN, wl1_t, wr1_t, b1_t, xT_t,
                      gathers_only=(mode == "l1g"))
                if mode == "l1g":
                    z = op.tile([1, F_OUT], f32, tag="z")
                    nc.vector.memset(z[:], 0.0)
                    nc.sync.dma_start(out_d[:1, :], z[:])
                elif mode == "l1":
                    # dump h1 pieces to out for inspection (fp16 -> fp32 on host)
                    for gph in range(AGP):
                        z = op.tile([1, F_OUT], f32, tag="z")
                        nc.vector.memset(z[:], 0.0)
                        nc.sync.dma_start(out_d[gph:gph + 1, :], z[:])
                elif mode == "l1+ag":
                    z = op.tile([1, F_OUT], f32, tag="z")
                    nc.vector.memset(z[:], 0.0)
                    nc.sync.dma_start(out_d[:1, :], z[:])
                else:
                    idx_t2, seg_t2, rsc_t2 = load_tables(2)
                    layer(2, idx_t2, seg_t2, rsc_t2, F_OUT, wl2_t, wr2_t, b2_t,
                          h1T_t)

    nc.finalize()
    return nc


def _make_in_maps(meta, x, W_l1, W_r1, b1, W_l2, W_r2, b2):
    x = np.asarray(x, dtype=np.float32)
    x_full = np.zeros((NG, F_IN), dtype=np.float32)
    for k in range(P):
        x_full[k * NL:k * NL + NREAL] = x[k * NREAL:(k + 1) * NREAL]
    x_fullh = x_full.astype(np.float16)
    iota = np.broadcast_to(np.arange(BLK, dtype=np.float16), (128, BLK)).copy()
    L1, L2 = meta["L1"], meta["L2"]
    in_maps = []
    for k in range(P):
        in_maps.append({
            "x_fullh": x_fullh,
            "xT": x_fullh[k * NL:(k + 1) * NL].T.copy(),
            "idx1": L1["idx"][k], "seg1": L1["seg"][k], "rsc1": L1["rsc"][k],
            "idx2": L2["idx"][k], "seg2": L2["seg"][k], "rsc2": L2["rsc"][k],
            "iota": iota,
            "W_l1": np.asarray(W_l1, np.float16),
            "W_r1": np.asarray(W_r1, np.float16),
            "b1": np.asarray(b1, np.float16).reshape(1, F_OUT),
            "W_l2": np.asarray(W_l2, np.float16),
            "W_r2": np.asarray(W_r2, np.float16),
            "b2": np.asarray(b2, np.float16).reshape(1, F_OUT),
        })
    return in_maps


def kernel(x, edge_index, W_l1, W_r1, b1, W_l2, W_r2, b2, _mode="full"):
    meta = _preprocess(np.asarray(edge_index))
    in_maps = _make_in_maps(meta, x, W_l1, W_r1, b1, W_l2, W_r2, b2)
    nc = _build(meta, mode=_mode)
    res = run_bass_kernel_spmd(nc, in_maps, core_ids=list(range(P)))
    out = np.concatenate(
        [res.results[k]["out"][:NREAL] for k in range(P)], axis=0)
    return out.astype(np.float32)


if __name__ == "__main__":
    rng = np.random.default_rng(0)
    n_small = None
    x = rng.normal(size=(N_NODES, F_IN)).astype(np.float32)
    ei = rng.integers(0, N_NODES, size=(2, N_EDGES)).astype(np.int64)
    wl1 = rng.normal(size=(F_IN, F_OUT)).astype(np.float32) / np.sqrt(F_IN)
    wr1 = rng.normal(size=(F_IN, F_OUT)).astype(np.float32) / np.sqrt(F_IN)
    wl2 = rng.normal(size=(F_OUT, F_OUT)).astype(np.float32) / np.sqrt(F_OUT)
    wr2 = rng.normal(size=(F_OUT, F_OUT)).astype(np.float32) / np.sqrt(F_OUT)
    b1 = np.zeros(F_OUT, np.float32)
    b2 = np.zeros(F_OUT, np.float32)
    out = kernel(x, ei, wl1, wr1, b1, wl2, wr2, b2)
    # numpy reference
    def ref():
        src, dst = ei[0], ei[1]
        def conv(h, Wl, Wr, b):
            sums = np.zeros((N_NODES, h.shape[1]), np.float32)
            np.add.at(sums, dst, h[src])
            cnt = np.bincount(dst, minlength=N_NODES).astype(np.float32)
            mean = sums / np.maximum(cnt, 1.0)[:, None]
            return mean @ Wl + h @ Wr + b
        h1 = np.maximum(conv(x, wl1, wr1, b1), 0.0)
        return conv(h1, wl2, wr2, b2)
    exp = ref()
    err = np.abs(out - exp)
    scale = np.abs(exp).max()
    print(f"absmax err {err.max():.4e} scale {scale:.3f} rel {err.max()/scale:.3e}")
